# revision 1
# baseline (speedup 1.0000x reference)
"""Trainium2 Bass kernel for a dense cross-task transformer block.

Math notes
----------
The reference "attention" has sequence length 1 on the key axis, so
softmax(scores) == 1.0 exactly and the whole q/k/score path is dead:

    mha_len1(q_in, kv_in, ...) == (kv_in @ wv.T + bv) @ wo.T + bo

which folds (on host) into a single matmul with W = wo @ wv and
b = wo @ bv + bo.  The block is then:

    verb1 = LN(verb + noun @ W1.T + c1)          (ln_v)
    verb2 = verb1 + FFN_v(verb1)
    noun1 = LN(noun + verb2 @ W2.T + c2)         (ln_n)
    noun2 = noun1 + FFN_n(noun1)
    return verb2, noun2

Device strategy
---------------
Pure data parallel over 8 cores (batch 16384 -> 2048 rows/core), weights
replicated.  On device everything is kept feature-major ([E, batch]) so
every matmul contracts along the SBUF partition dim.  Matmuls run as
float32r (1 cycle/row for N>=256).  LayerNorm reduces across partitions
via ones-vector matmuls; stats are broadcast back across partitions with
K=1 matmuls.  The second FFN matmul runs in bf16 (hidden activations and
w2 weights) - the result only feeds a residual delta, so precision loss
is negligible.
"""

import numpy as np
import ml_dtypes
from contextlib import ExitStack

import concourse.bass as bass
import concourse.bacc as bacc_mod
import concourse.mybir as mybir
import concourse.tile as tile
from concourse.bass_utils import run_bass_kernel_spmd

E = 1024          # embed dim
H2 = 2048         # FFN hidden dim
B_TOTAL = 16384
NCORES = 8
B = B_TOTAL // NCORES   # 2048 rows per core
P = 128
EPS = 1e-5
CHUNK = 512       # attn/LN phase column chunk
NCHUNKS = B // CHUNK
KT = E // P       # 8  k-tiles over E
MT = E // P       # 8  m-tiles over E
HT = H2 // P      # 16 tiles over hidden

F32 = mybir.dt.float32
F32R = mybir.dt.float32r
BF16 = mybir.dt.bfloat16
AF = mybir.ActivationFunctionType
OP = mybir.AluOpType


def _load_pvec(nc, pool, dram_ap, ntiles, tag):
    """DRAM [ntiles*128] vector -> SBUF [128, ntiles], element (p,t) = v[t*128+p]."""
    t = pool.tile([P, ntiles], F32, tag=tag, name=tag)
    nc.sync.dma_start(out=t[:], in_=dram_ap.rearrange("(t p) -> p t", p=P))
    return t


def _build_program():
    nc = bacc_mod.Bacc("TRN2", target_bir_lowering=False)

    vT = nc.declare_dram_parameter("vT", [E, B], F32, isOutput=False)
    nT = nc.declare_dram_parameter("nT", [E, B], F32, isOutput=False)
    wvo1 = nc.declare_dram_parameter("wvo1", [E, E], F32, isOutput=False)     # (wo@wv).T : [k, m]
    bvo1 = nc.declare_dram_parameter("bvo1", [E], F32, isOutput=False)
    wvo2 = nc.declare_dram_parameter("wvo2", [E, E], F32, isOutput=False)
    bvo2 = nc.declare_dram_parameter("bvo2", [E], F32, isOutput=False)
    lnvg = nc.declare_dram_parameter("lnvg", [E], F32, isOutput=False)
    lnvb = nc.declare_dram_parameter("lnvb", [E], F32, isOutput=False)
    lnng = nc.declare_dram_parameter("lnng", [E], F32, isOutput=False)
    lnnb = nc.declare_dram_parameter("lnnb", [E], F32, isOutput=False)
    w1v = nc.declare_dram_parameter("w1v", [E, H2], F32, isOutput=False)      # fv_w1.T
    b1v = nc.declare_dram_parameter("b1v", [H2], F32, isOutput=False)
    w2v = nc.declare_dram_parameter("w2v", [H2, E], BF16, isOutput=False)     # fv_w2.T in bf16
    b2v = nc.declare_dram_parameter("b2v", [E], F32, isOutput=False)
    w1n = nc.declare_dram_parameter("w1n", [E, H2], F32, isOutput=False)
    b1n = nc.declare_dram_parameter("b1n", [H2], F32, isOutput=False)
    w2n = nc.declare_dram_parameter("w2n", [H2, E], BF16, isOutput=False)
    b2n = nc.declare_dram_parameter("b2n", [E], F32, isOutput=False)
    ones_d = nc.declare_dram_parameter("ones_d", [P, 1], F32, isOutput=False)
    verb_out = nc.declare_dram_parameter("verb_out", [E, B], F32, isOutput=True)
    noun_out = nc.declare_dram_parameter("noun_out", [E, B], F32, isOutput=True)

    with tile.TileContext(nc) as tc, ExitStack() as ctx:
        const = ctx.enter_context(tc.tile_pool(name="const", bufs=1))
        resid = ctx.enter_context(tc.tile_pool(name="resid", bufs=1))

        ones_col = const.tile([P, 1], F32R, tag="ones_col", name="ones_col")
        nc.sync.dma_start(out=ones_col[:], in_=ones_d[:, :].bitcast(F32R))
        ones_row = const.tile([1, P], F32, tag="ones_row", name="ones_row")
        nc.vector.memset(ones_row[:], 1.0)
        eps_t = const.tile([1, 1], F32, tag="eps", name="eps")
        nc.vector.memset(eps_t[:], EPS)

        bvo1_pb = _load_pvec(nc, const, bvo1[:], MT, "bvo1")
        bvo2_pb = _load_pvec(nc, const, bvo2[:], MT, "bvo2")
        lnvg_pb = _load_pvec(nc, const, lnvg[:], MT, "lnvg")
        lnvb_pb = _load_pvec(nc, const, lnvb[:], MT, "lnvb")
        lnng_pb = _load_pvec(nc, const, lnng[:], MT, "lnng")
        lnnb_pb = _load_pvec(nc, const, lnnb[:], MT, "lnnb")
        b1v_pb = _load_pvec(nc, const, b1v[:], HT, "b1v")
        b2v_pb = _load_pvec(nc, const, b2v[:], MT, "b2v")
        b1n_pb = _load_pvec(nc, const, b1n[:], HT, "b1n")
        b2n_pb = _load_pvec(nc, const, b2n[:], MT, "b2n")

        # persistent residual-stream tiles ([128, B] f32); verb1 in phases
        # A/B, overwritten as noun1 in phases C/D (same tags -> same slots)
        def resid_tiles():
            return [resid.tile([P, B], F32R, tag=f"r{m}", name=f"r{m}") for m in range(MT)]

        def attn_ln_phase(sfx, kxn_dram, res_dram, w_dram, bias_pb, g_pb, b_pb):
            """out_tiles[m][:, :] = LN(res + kxn.T @ w + bias) feature-major."""
            out_tiles = resid_tiles()
            with ExitStack() as pctx:
                wpool = pctx.enter_context(tc.tile_pool(name=f"wv{sfx}", bufs=1))
                kxp = pctx.enter_context(tc.tile_pool(name=f"kx{sfx}", bufs=1))
                vp = pctx.enter_context(tc.tile_pool(name=f"vp{sfx}", bufs=2))
                sqp = pctx.enter_context(tc.tile_pool(name=f"sq{sfx}", bufs=2))
                sm = pctx.enter_context(tc.tile_pool(name=f"sm{sfx}", bufs=1))
                aps = pctx.enter_context(
                    tc.tile_pool(name=f"aps{sfx}", bufs=2, space="PSUM"))
                stp = pctx.enter_context(
                    tc.tile_pool(name=f"st{sfx}", bufs=1, space="PSUM"))
                bcp = pctx.enter_context(
                    tc.tile_pool(name=f"bc{sfx}", bufs=1, space="PSUM"))

                w_tiles = []
                for k in range(KT):
                    wt = wpool.tile([P, E], F32R, tag=f"w{k}", name=f"w{k}")
                    nc.sync.dma_start(out=wt[:], in_=w_dram[k * P:(k + 1) * P, :].bitcast(F32R))
                    w_tiles.append(wt)

                for c in range(NCHUNKS):
                    cs = slice(c * CHUNK, (c + 1) * CHUNK)
                    kx = []
                    for k in range(KT):
                        t = kxp.tile([P, CHUNK], F32R, tag=f"k{k}", name=f"k{k}")
                        nc.sync.dma_start(out=t[:], in_=kxn_dram[k * P:(k + 1) * P, cs].bitcast(F32R))
                        kx.append(t)
                    stats_x = stp.tile([1, CHUNK], F32, tag="sx", name="sx")
                    stats_q = stp.tile([1, CHUNK], F32, tag="sq", name="sq")
                    for m in range(MT):
                        ps = aps.tile([P, CHUNK], F32, tag="ps", name="ps")
                        for k in range(KT):
                            nc.tensor.matmul(
                                ps[:],
                                lhsT=w_tiles[k][:, m * P:(m + 1) * P],
                                rhs=kx[k][:],
                                start=(k == 0), stop=(k == KT - 1))
                        vt = vp.tile([P, CHUNK], F32, tag="v", name="v")
                        nc.sync.dma_start(out=vt[:], in_=res_dram[m * P:(m + 1) * P, cs])
                        xt = out_tiles[m][:, cs]
                        nc.vector.tensor_add(xt, ps[:], vt[:])
                        nc.vector.tensor_scalar(
                            xt, xt, bias_pb[:, m:m + 1], None, OP.add)
                        sq = sqp.tile([P, CHUNK], F32R, tag="s", name="s")
                        nc.scalar.activation(sq[:], xt, AF.Square)
                        nc.tensor.matmul(stats_x[:], lhsT=ones_col[:],
                                         rhs=xt,
                                         start=(m == 0), stop=(m == MT - 1))
                        nc.tensor.matmul(stats_q[:], lhsT=ones_col[:],
                                         rhs=sq[:],
                                         start=(m == 0), stop=(m == MT - 1))
                    # column stats -> -mean, 1/std  ([1, CHUNK])
                    nm = sm.tile([1, CHUNK], F32, tag="nm", name="nm")
                    nc.scalar.activation(nm[:], stats_x[:], AF.Copy, scale=-1.0 / E)
                    t1 = sm.tile([1, CHUNK], F32, tag="t1", name="t1")
                    nc.scalar.activation(t1[:], stats_q[:], AF.Copy, scale=1.0 / E)
                    m2 = sm.tile([1, CHUNK], F32, tag="m2", name="m2")
                    nc.vector.tensor_mul(m2[:], nm[:], nm[:])
                    nc.vector.tensor_sub(t1[:], t1[:], m2[:])          # var
                    nc.scalar.activation(t1[:], t1[:], AF.Sqrt, bias=eps_t[:])
                    rs = sm.tile([1, CHUNK], F32, tag="rs", name="rs")
                    nc.vector.reciprocal(rs[:], t1[:])
                    # broadcast across partitions via K=1 matmuls (exact fp32)
                    nmB = bcp.tile([P, CHUNK], F32, tag="nmB", name="nmB")
                    nc.tensor.matmul(nmB[:], lhsT=ones_row[:], rhs=nm[:],
                                     start=True, stop=True)
                    rsB = bcp.tile([P, CHUNK], F32, tag="rsB", name="rsB")
                    nc.tensor.matmul(rsB[:], lhsT=ones_row[:], rhs=rs[:],
                                     start=True, stop=True)
                    for m in range(MT):
                        xt = out_tiles[m][:, cs]
                        nc.vector.tensor_add(xt, xt, nmB[:])
                        nc.vector.tensor_mul(xt, xt, rsB[:])
                        nc.vector.tensor_scalar(
                            xt, xt, g_pb[:, m:m + 1], b_pb[:, m:m + 1],
                            OP.mult, OP.add)
            return out_tiles

        def ffn_phase(sfx, in_tiles, h_tiles, w1_dram, b1_pb, w2_dram, b2_pb,
                      out_dram):
            """out = in + W2.T@gelu(W1.T@in + b1) + b2; streams to out_dram."""
            with ExitStack() as pctx:
                w1p = pctx.enter_context(tc.tile_pool(name=f"w1{sfx}", bufs=4))
                w2p = pctx.enter_context(tc.tile_pool(name=f"w2{sfx}", bufs=4))
                op = pctx.enter_context(tc.tile_pool(name=f"op{sfx}", bufs=2))
                fps = pctx.enter_context(
                    tc.tile_pool(name=f"fps{sfx}", bufs=2, space="PSUM"))
                for hm in range(HT):
                    ps = fps.tile([P, B], F32, tag="f", name="f")
                    for k in range(KT):
                        wt = w1p.tile([P, P], F32R, tag="w", name="w")
                        nc.sync.dma_start(
                            out=wt[:], in_=w1_dram[k * P:(k + 1) * P,
                                                   hm * P:(hm + 1) * P].bitcast(F32R))
                        for ns in range(B // 512):
                            nss = slice(ns * 512, (ns + 1) * 512)
                            nc.tensor.matmul(
                                ps[:, nss], lhsT=wt[:],
                                rhs=in_tiles[k][:, nss],
                                start=(k == 0), stop=(k == KT - 1))
                    nc.scalar.activation(h_tiles[hm][:], ps[:], AF.Gelu,
                                         bias=b1_pb[:, hm:hm + 1])
                for m in range(MT):
                    ps = fps.tile([P, B], F32, tag="f", name="f")
                    for k in range(HT):
                        wt = w2p.tile([P, P], BF16, tag="w", name="w")
                        nc.sync.dma_start(
                            out=wt[:], in_=w2_dram[k * P:(k + 1) * P,
                                                   m * P:(m + 1) * P])
                        for ns in range(B // 512):
                            nss = slice(ns * 512, (ns + 1) * 512)
                            nc.tensor.matmul(
                                ps[:, nss], lhsT=wt[:],
                                rhs=h_tiles[k][:, nss],
                                start=(k == 0), stop=(k == HT - 1))
                    ot = op.tile([P, B], F32, tag="o", name="o")
                    nc.vector.tensor_add(ot[:], ps[:], in_tiles[m][:])
                    nc.vector.tensor_scalar(
                        ot[:], ot[:], b2_pb[:, m:m + 1], None, OP.add)
                    nc.sync.dma_start(out=out_dram[m * P:(m + 1) * P, :], in_=ot[:])

        import os as _os
        _REP = int(_os.environ.get("BENCH_REPEAT", "1"))
        with ExitStack() as hctx:
            hp = hctx.enter_context(tc.tile_pool(name="hbf", bufs=1))

            def h_tiles():
                return [hp.tile([P, B], BF16, tag=f"h{i}", name=f"h{i}") for i in range(HT)]

            for _rep in range(_REP):
                # phase A: verb attends to noun, LN -> verb1 (resident)
                verb1 = attn_ln_phase(f"a{_rep}", nT, vT, wvo1, bvo1_pb,
                                      lnvg_pb, lnvb_pb)
                # phase B: verb FFN -> verb_out (DRAM)
                ffn_phase(f"b{_rep}", verb1, h_tiles(), w1v, b1v_pb, w2v,
                          b2v_pb, verb_out)
                # phase C: noun attends to verb2 (read back), LN -> noun1
                noun1 = attn_ln_phase(f"c{_rep}", verb_out, nT, wvo2, bvo2_pb,
                                      lnng_pb, lnnb_pb)
                # phase D: noun FFN -> noun_out
                ffn_phase(f"d{_rep}", noun1, h_tiles(), w1n, b1n_pb, w2n,
                          b2n_pb, noun_out)

    nc.finalize()
    return nc


_prog_cache = {}


def _get_program():
    if "nc" not in _prog_cache:
        _prog_cache["nc"] = _build_program()
    return _prog_cache["nc"]


def _prepare_maps(inputs):
    f32 = np.float32
    g = {k: np.asarray(v, f32) for k, v in inputs.items()}

    def fold(p):
        w = g[f"{p}_wo"] @ g[f"{p}_wv"]
        b = g[f"{p}_wo"] @ g[f"{p}_bv"] + g[f"{p}_bo"]
        return np.ascontiguousarray(w.T), np.ascontiguousarray(b)

    wvo1, bvo1 = fold("v2n")
    wvo2, bvo2 = fold("n2v")
    common = {
        "wvo1": wvo1, "bvo1": bvo1, "wvo2": wvo2, "bvo2": bvo2,
        "lnvg": g["ln_v_g"], "lnvb": g["ln_v_b"],
        "lnng": g["ln_n_g"], "lnnb": g["ln_n_b"],
        "w1v": np.ascontiguousarray(g["fv_w1"].T), "b1v": g["fv_b1"],
        "w2v": np.ascontiguousarray(g["fv_w2"].T).astype(ml_dtypes.bfloat16),
        "b2v": g["fv_b2"],
        "w1n": np.ascontiguousarray(g["fn_w1"].T), "b1n": g["fn_b1"],
        "w2n": np.ascontiguousarray(g["fn_w2"].T).astype(ml_dtypes.bfloat16),
        "b2n": g["fn_b2"],
        "ones_d": np.ones((128, 1), f32),
    }
    vT = np.ascontiguousarray(g["verb_features"].T)   # [E, 16384]
    nT = np.ascontiguousarray(g["noun_features"].T)
    in_maps = []
    for i in range(NCORES):
        cs = slice(i * B, (i + 1) * B)
        m = dict(common)
        m["vT"] = np.ascontiguousarray(vT[:, cs])
        m["nT"] = np.ascontiguousarray(nT[:, cs])
        in_maps.append(m)
    return in_maps


def kernel(**inputs):
    nc = _get_program()
    in_maps = _prepare_maps(inputs)
    res = run_bass_kernel_spmd(nc, in_maps, list(range(NCORES))).results
    verb = np.concatenate([res[i]["verb_out"] for i in range(NCORES)], axis=1)
    noun = np.concatenate([res[i]["noun_out"] for i in range(NCORES)], axis=1)
    return np.ascontiguousarray(verb.T), np.ascontiguousarray(noun.T)



# revision 9
# speedup vs baseline: 1.1767x; 1.1767x over previous
"""Trainium2 Bass kernel for a dense cross-task transformer block.

Math notes
----------
The reference "attention" has sequence length 1 on the key axis, so
softmax(scores) == 1.0 exactly and the whole q/k/score path is dead:

    mha_len1(q_in, kv_in, ...) == (kv_in @ wv.T + bv) @ wo.T + bo

which folds (on host) into a single matmul with W = wo @ wv and
b = wo @ bv + bo.  The block is then:

    verb1 = LN(verb + noun @ W1.T + c1)          (ln_v)
    verb2 = verb1 + FFN_v(verb1)
    noun1 = LN(noun + verb2 @ W2.T + c2)         (ln_n)
    noun2 = noun1 + FFN_n(noun1)
    return verb2, noun2

Device strategy
---------------
Pure data parallel over 8 cores (batch 16384 -> 2048 cols/core), weights
replicated, everything feature-major ([E, batch]) so matmuls contract
along the SBUF partition dim.

v2 design (vs the f32r baseline):
 - all matmul operands in bf16 (fp32 PSUM accumulation) -> FastWeightLoad
   kicks in and LDWEIGHTS overlaps fully; inputs/weights uploaded bf16.
 - phases stream per 512-column chunk; verb2 stays resident in SBUF for
   phase C (no DRAM round trip).
 - FFN loops are chunk-outer so FFN chunk c starts as soon as the LN of
   chunk c lands; w1 resident per phase, w2 streamed per chunk-halfblock.
 - single 8-bank PSUM pool choreography: mains/ffn1 rotate banks 0-3,
   attn stats use 4-5, LN broadcasts 6-7, ffn2 half-blocks use 0-3/4-7.
 - LayerNorm reduces across partitions via ones-vector matmuls; -mean and
   1/std broadcast back with K=1 matmuls; scale/shift fused in one
   tensor_scalar op.
"""

import os
import numpy as np
import ml_dtypes
from contextlib import ExitStack

import concourse.bass as bass
import concourse.bacc as bacc_mod
import concourse.mybir as mybir
import concourse.tile as tile
from concourse.bass_utils import run_bass_kernel_spmd

E = 1024          # embed dim
H2 = 2048         # FFN hidden dim
B_TOTAL = 16384
NCORES = 8
B = B_TOTAL // NCORES   # 2048 cols per core
P = 128
EPS = 1e-5
CHUNK = 512
NCH = B // CHUNK  # 4
KT = E // P       # 8
MT = E // P       # 8
HT = H2 // P      # 16

F32 = mybir.dt.float32
F32R = mybir.dt.float32r
BF16 = mybir.dt.bfloat16
AF = mybir.ActivationFunctionType
OP = mybir.AluOpType


def _build_program():
    nc = bacc_mod.Bacc("TRN2", target_bir_lowering=False)

    vT = nc.declare_dram_parameter("vT", [E, B], BF16, isOutput=False)
    nT = nc.declare_dram_parameter("nT", [E, B], BF16, isOutput=False)
    wat1 = nc.declare_dram_parameter("wat1", [E, E], BF16, isOutput=False)  # (wo@wv).T
    wat2 = nc.declare_dram_parameter("wat2", [E, E], BF16, isOutput=False)
    bat1 = nc.declare_dram_parameter("bat1", [E], F32, isOutput=False)
    bat2 = nc.declare_dram_parameter("bat2", [E], F32, isOutput=False)
    lnvg = nc.declare_dram_parameter("lnvg", [E], F32, isOutput=False)
    lnvb = nc.declare_dram_parameter("lnvb", [E], F32, isOutput=False)
    lnng = nc.declare_dram_parameter("lnng", [E], F32, isOutput=False)
    lnnb = nc.declare_dram_parameter("lnnb", [E], F32, isOutput=False)
    w1v = nc.declare_dram_parameter("w1v", [E, H2], BF16, isOutput=False)   # fv_w1.T
    b1v = nc.declare_dram_parameter("b1v", [H2], F32, isOutput=False)
    w2v = nc.declare_dram_parameter("w2v", [H2, E], BF16, isOutput=False)   # fv_w2.T
    b2v = nc.declare_dram_parameter("b2v", [E], F32, isOutput=False)
    w1n = nc.declare_dram_parameter("w1n", [E, H2], BF16, isOutput=False)
    b1n = nc.declare_dram_parameter("b1n", [H2], F32, isOutput=False)
    w2n = nc.declare_dram_parameter("w2n", [H2, E], BF16, isOutput=False)
    b2n = nc.declare_dram_parameter("b2n", [E], F32, isOutput=False)
    ones_f = nc.declare_dram_parameter("ones_f", [P, 1], F32, isOutput=False)
    ones_fr_d = nc.declare_dram_parameter("ones_fr_d", [1, P], F32, isOutput=False)
    ones_b_d = nc.declare_dram_parameter("ones_b_d", [P, 1], BF16, isOutput=False)
    verb_out = nc.declare_dram_parameter("verb_out", [E, B], F32, isOutput=True)
    noun_out = nc.declare_dram_parameter("noun_out", [E, B], F32, isOutput=True)

    with tile.TileContext(nc) as tc, ExitStack() as ctx:
        const = ctx.enter_context(tc.tile_pool(name="const", bufs=1))
        kxp = ctx.enter_context(tc.tile_pool(name="kxp", bufs=2))
        rp = ctx.enter_context(tc.tile_pool(name="rp", bufs=2))
        v1p = ctx.enter_context(tc.tile_pool(name="v1p", bufs=1))
        v2p = ctx.enter_context(tc.tile_pool(name="v2p", bufs=1))
        hp = ctx.enter_context(tc.tile_pool(name="hp", bufs=2))
        wap = ctx.enter_context(tc.tile_pool(name="wap", bufs=1))
        w1p = ctx.enter_context(tc.tile_pool(name="w1p", bufs=1))
        w2sp = ctx.enter_context(tc.tile_pool(name="w2sp", bufs=3))
        stp = ctx.enter_context(tc.tile_pool(name="stp", bufs=4))
        sqp = ctx.enter_context(tc.tile_pool(name="sqp", bufs=2))
        smp = ctx.enter_context(tc.tile_pool(name="smp", bufs=1))
        psp = ctx.enter_context(tc.tile_pool(name="psp", bufs=1, space="PSUM"))

        def psum(i):
            return psp.tile([P, CHUNK], F32, tag=f"b{i}", name=f"b{i}")

        def psum_stat(i):
            return psp.tile([1, CHUNK], F32, tag=f"b{i}", name=f"b{i}")

        ones_bf = const.tile([P, 1], BF16, tag="ones_bf", name="ones_bf")
        nc.sync.dma_start(out=ones_bf[:], in_=ones_b_d[:, :])
        ones_fr = const.tile([P, 1], F32R, tag="ones_fr", name="ones_fr")
        nc.sync.dma_start(out=ones_fr[:], in_=ones_f[:, :].bitcast(F32R))
        onesrow_fr = const.tile([1, P], F32R, tag="onesrow", name="onesrow")
        nc.sync.dma_start(out=onesrow_fr[:], in_=ones_fr_d[:, :].bitcast(F32R))
        eps_t = const.tile([1, 1], F32, tag="eps", name="eps")
        nc.vector.memset(eps_t[:], EPS)

        def load_pvec(dram_ap, ntiles, tag):
            t = const.tile([P, ntiles], F32, tag=tag, name=tag)
            nc.sync.dma_start(out=t[:], in_=dram_ap.rearrange("(t p) -> p t", p=P))
            return t

        bat1_pb = load_pvec(bat1[:], MT, "bat1")
        bat2_pb = load_pvec(bat2[:], MT, "bat2")
        lnvg_pb = load_pvec(lnvg[:], MT, "lnvg")
        lnvb_pb = load_pvec(lnvb[:], MT, "lnvb")
        lnng_pb = load_pvec(lnng[:], MT, "lnng")
        lnnb_pb = load_pvec(lnnb[:], MT, "lnnb")
        b1v_pb = load_pvec(b1v[:], HT, "b1v")
        b2v_pb = load_pvec(b2v[:], MT, "b2v")
        b1n_pb = load_pvec(b1n[:], HT, "b1n")
        b2n_pb = load_pvec(b2n[:], MT, "b2n")

        def load_attn_w(w_dram):
            tiles = []
            for k in range(KT):
                t = wap.tile([P, E], BF16, tag=f"aw{k}", name=f"aw{k}")
                nc.sync.dma_start(out=t[:], in_=w_dram[k * P:(k + 1) * P, :])
                tiles.append(t)
            return tiles

        def load_w1(w_dram):
            tiles = []
            for k in range(KT):
                t = w1p.tile([P, H2], BF16, tag=f"w1_{k}", name=f"w1_{k}")
                nc.sync.dma_start(out=t[:], in_=w_dram[k * P:(k + 1) * P, :])
                tiles.append(t)
            return tiles

        def attn_ln(wt, kx_dram, kx_tiles, res_dram, bias_pb, g_pb, b_pb,
                    out_tiles):
            """out[m][:,cs] = LN(res + W@kx + bias) for each 512-col chunk."""
            for c in range(NCH):
                cs = slice(c * CHUNK, (c + 1) * CHUNK)
                if kx_tiles is None:
                    kx = []
                    for k in range(KT):
                        t = kxp.tile([P, CHUNK], BF16, tag=f"kx{k}", name=f"kx{k}")
                        nc.sync.dma_start(out=t[:], in_=kx_dram[k * P:(k + 1) * P, cs])
                        kx.append(t[:])
                else:
                    kx = [kx_tiles[k][:, cs] for k in range(KT)]
                rt = []
                for m in range(MT):
                    t = rp.tile([P, CHUNK], BF16, tag=f"r{m}", name=f"r{m}")
                    nc.sync.dma_start(out=t[:], in_=res_dram[m * P:(m + 1) * P, cs])
                    rt.append(t)
                stats_x = psum_stat(4)
                stats_q = psum_stat(5)
                for m in range(MT):
                    ps = psum(m % 4)
                    for k in range(KT):
                        nc.tensor.matmul(
                            ps[:], lhsT=wt[k][:, m * P:(m + 1) * P],
                            rhs=kx[k],
                            start=(k == 0), stop=(k == KT - 1))
                    xt = out_tiles[m][:, cs]
                    nc.vector.tensor_scalar(
                        xt, ps[:], bias_pb[:, m:m + 1], None, OP.add)
                    nc.vector.tensor_add(xt, xt, rt[m][:])
                    sqm = sqp.tile([P, CHUNK], F32R, tag="sq", name="sq")
                    nc.scalar.activation(sqm[:], xt, AF.Square)
                    nc.tensor.matmul(stats_x[:], lhsT=ones_bf[:], rhs=xt,
                                     start=(m == 0), stop=(m == MT - 1))
                    nc.tensor.matmul(stats_q[:], lhsT=ones_fr[:], rhs=sqm[:],
                                     start=(m == 0), stop=(m == MT - 1))
                # column stats -> -mean, 1/std  ([1, CHUNK])
                nm = smp.tile([1, CHUNK], F32R, tag="nm", name="nm")
                nc.scalar.activation(nm[:], stats_x[:], AF.Copy, scale=-1.0 / E)
                t1 = smp.tile([1, CHUNK], F32, tag="t1", name="t1")
                nc.scalar.activation(t1[:], stats_q[:], AF.Copy, scale=1.0 / E)
                m2 = smp.tile([1, CHUNK], F32, tag="m2", name="m2")
                nc.vector.tensor_mul(m2[:], nm[:], nm[:])
                nc.vector.tensor_sub(t1[:], t1[:], m2[:])           # var
                nc.scalar.activation(t1[:], t1[:], AF.Sqrt, bias=eps_t[:])
                rs = smp.tile([1, CHUNK], F32R, tag="rs", name="rs")
                with nc.allow_low_precision(reason="f32r is bit-identical to f32"):
                    nc.vector.reciprocal(rs[:], t1[:])
                nmB = psum(6)
                nc.tensor.matmul(nmB[:], lhsT=onesrow_fr[:], rhs=nm[:],
                                 start=True, stop=True)
                rsB = psum(7)
                nc.tensor.matmul(rsB[:], lhsT=onesrow_fr[:], rhs=rs[:],
                                 start=True, stop=True)
                for m in range(MT):
                    xt = out_tiles[m][:, cs]
                    nc.vector.tensor_add(xt, xt, nmB[:])
                    nc.vector.tensor_mul(xt, xt, rsB[:])
                    nc.vector.tensor_scalar(
                        xt, xt, g_pb[:, m:m + 1], b_pb[:, m:m + 1],
                        OP.mult, OP.add)

        def ffn(in_tiles, w1t, b1_pb, w2_dram, b2_pb, out_dram, out_bf):
            """out = in + W2.T@gelu(W1.T@in + b1) + b2, per 512-col chunk."""
            for c in range(NCH):
                cs = slice(c * CHUNK, (c + 1) * CHUNK)
                hts = []
                for hm in range(HT):
                    ps = psum(hm % 4)
                    for k in range(KT):
                        nc.tensor.matmul(
                            ps[:], lhsT=w1t[k][:, hm * P:(hm + 1) * P],
                            rhs=in_tiles[k][:, cs],
                            start=(k == 0), stop=(k == KT - 1))
                    ht = hp.tile([P, CHUNK], BF16, tag=f"h{hm}", name=f"h{hm}")
                    nc.scalar.activation(ht[:], ps[:], AF.Gelu,
                                         bias=b1_pb[:, hm:hm + 1])
                    hts.append(ht)
                for blk in range(2):
                    ms = range(blk * 4, blk * 4 + 4)
                    pss = [psum(blk * 4 + mi) for mi in range(4)]
                    for k in range(HT):
                        w2t = w2sp.tile([P, CHUNK], BF16, tag="w2s", name="w2s")
                        nc.sync.dma_start(
                            out=w2t[:],
                            in_=w2_dram[k * P:(k + 1) * P,
                                        blk * CHUNK:(blk + 1) * CHUNK])
                        for mi, m in enumerate(ms):
                            nc.tensor.matmul(
                                pss[mi][:], lhsT=w2t[:, mi * P:(mi + 1) * P],
                                rhs=hts[k][:],
                                start=(k == 0), stop=(k == HT - 1))
                    for mi, m in enumerate(ms):
                        st = stp.tile([P, CHUNK], F32, tag="st", name="st")
                        nc.vector.tensor_add(st[:], pss[mi][:],
                                             in_tiles[m][:, cs])
                        nc.scalar.activation(st[:], st[:], AF.Identity,
                                             bias=b2_pb[:, m:m + 1])
                        nc.sync.dma_start(out=out_dram[m * P:(m + 1) * P, cs],
                                          in_=st[:])
                        if out_bf is not None:
                            nc.scalar.activation(out_bf[m][:, cs], st[:],
                                                 AF.Copy)

        _REP = int(os.environ.get("BENCH_REPEAT", "1"))
        for _rep in range(_REP):
            # A: verb attends to noun, LN -> verb1 (SBUF resident)
            wA = load_attn_w(wat1)
            w1tv = load_w1(w1v)
            v1 = [v1p.tile([P, B], BF16, tag=f"v1_{m}", name=f"v1_{m}")
                  for m in range(MT)]
            attn_ln(wA, nT, None, vT, bat1_pb, lnvg_pb, lnvb_pb, v1)
            # B: verb FFN -> verb_out (DRAM, f32) + verb2 (SBUF bf16)
            v2 = [v2p.tile([P, B], BF16, tag=f"v2_{m}", name=f"v2_{m}")
                  for m in range(MT)]
            ffn(v1, w1tv, b1v_pb, w2v, b2v_pb, verb_out, v2)
            # C: noun attends to verb2 (SBUF), LN -> noun1 (reuses v1 slots)
            wC = load_attn_w(wat2)
            w1tn = load_w1(w1n)
            n1 = [v1p.tile([P, B], BF16, tag=f"v1_{m}", name=f"v1_{m}")
                  for m in range(MT)]
            attn_ln(wC, None, v2, nT, bat2_pb, lnng_pb, lnnb_pb, n1)
            # D: noun FFN -> noun_out
            ffn(n1, w1tn, b1n_pb, w2n, b2n_pb, noun_out, None)

    nc.finalize()
    return nc


_prog_cache = {}


def _get_program():
    if "nc" not in _prog_cache:
        _prog_cache["nc"] = _build_program()
    return _prog_cache["nc"]


def _prepare_maps(inputs):
    f32 = np.float32
    bf16 = ml_dtypes.bfloat16
    g = {k: np.asarray(v, f32) for k, v in inputs.items()}

    def fold(p):
        w = g[f"{p}_wo"] @ g[f"{p}_wv"]
        b = g[f"{p}_wo"] @ g[f"{p}_bv"] + g[f"{p}_bo"]
        return np.ascontiguousarray(w.T).astype(bf16), np.ascontiguousarray(b)

    wat1, bat1 = fold("v2n")
    wat2, bat2 = fold("n2v")
    common = {
        "wat1": wat1, "bat1": bat1, "wat2": wat2, "bat2": bat2,
        "lnvg": g["ln_v_g"], "lnvb": g["ln_v_b"],
        "lnng": g["ln_n_g"], "lnnb": g["ln_n_b"],
        "w1v": np.ascontiguousarray(g["fv_w1"].T).astype(bf16), "b1v": g["fv_b1"],
        "w2v": np.ascontiguousarray(g["fv_w2"].T).astype(bf16), "b2v": g["fv_b2"],
        "w1n": np.ascontiguousarray(g["fn_w1"].T).astype(bf16), "b1n": g["fn_b1"],
        "w2n": np.ascontiguousarray(g["fn_w2"].T).astype(bf16), "b2n": g["fn_b2"],
        "ones_f": np.ones((P, 1), f32),
        "ones_fr_d": np.ones((1, P), f32),
        "ones_b_d": np.ones((P, 1), bf16),
    }
    vT = np.ascontiguousarray(g["verb_features"].T).astype(bf16)  # [E, 16384]
    nT = np.ascontiguousarray(g["noun_features"].T).astype(bf16)
    in_maps = []
    for i in range(NCORES):
        cs = slice(i * B, (i + 1) * B)
        m = dict(common)
        m["vT"] = np.ascontiguousarray(vT[:, cs])
        m["nT"] = np.ascontiguousarray(nT[:, cs])
        in_maps.append(m)
    return in_maps


def kernel(**inputs):
    nc = _get_program()
    in_maps = _prepare_maps(inputs)
    res = run_bass_kernel_spmd(nc, in_maps, list(range(NCORES))).results
    verb = np.concatenate([res[i]["verb_out"] for i in range(NCORES)], axis=1)
    noun = np.concatenate([res[i]["noun_out"] for i in range(NCORES)], axis=1)
    return np.ascontiguousarray(verb.T), np.ascontiguousarray(noun.T)


# revision 15
# speedup vs baseline: 1.2838x; 1.0910x over previous
"""Trainium2 Bass kernel for a dense cross-task transformer block.

Math notes
----------
The reference "attention" has sequence length 1 on the key axis, so
softmax(scores) == 1.0 exactly and the whole q/k/score path is dead:

    mha_len1(q_in, kv_in, ...) == (kv_in @ wv.T + bv) @ wo.T + bo

which folds (on host) into a single matmul with W = wo @ wv and
b = wo @ bv + bo.  The block is then:

    verb1 = LN(verb + noun @ W1.T + c1)          (ln_v)
    verb2 = verb1 + FFN_v(verb1)
    noun1 = LN(noun + verb2 @ W2.T + c2)         (ln_n)
    noun2 = noun1 + FFN_n(noun1)
    return verb2, noun2

Device strategy
---------------
Pure data parallel over 8 cores (batch 16384 -> 2048 cols/core), weights
replicated, everything feature-major ([E, batch]) so matmuls contract
along the SBUF partition dim.

v2 design (vs the f32r baseline):
 - all matmul operands in bf16 (fp32 PSUM accumulation) -> FastWeightLoad
   kicks in and LDWEIGHTS overlaps fully; inputs/weights uploaded bf16.
 - phases stream per 512-column chunk; verb2 stays resident in SBUF for
   phase C (no DRAM round trip).
 - FFN loops are chunk-outer so FFN chunk c starts as soon as the LN of
   chunk c lands; w1 resident per phase, w2 streamed per chunk-halfblock.
 - single 8-bank PSUM pool choreography: mains/ffn1 rotate banks 0-3,
   attn stats use 4-5, LN broadcasts 6-7, ffn2 half-blocks use 0-3/4-7.
 - LayerNorm reduces across partitions via ones-vector matmuls; -mean and
   1/std broadcast back with K=1 matmuls; scale/shift fused in one
   tensor_scalar op.
"""

import os
import numpy as np
import ml_dtypes
from contextlib import ExitStack

import concourse.bass as bass
import concourse.bacc as bacc_mod
import concourse.mybir as mybir
import concourse.tile as tile
from concourse.bass_utils import run_bass_kernel_spmd

E = 1024          # embed dim
H2 = 2048         # FFN hidden dim
B_TOTAL = 16384
NCORES = 8
B = B_TOTAL // NCORES   # 2048 cols per core
P = 128
EPS = 1e-5
CHUNK = 512
NCH = B // CHUNK  # 4
KT = E // P       # 8
MT = E // P       # 8
HT = H2 // P      # 16

F32 = mybir.dt.float32
F32R = mybir.dt.float32r
BF16 = mybir.dt.bfloat16
AF = mybir.ActivationFunctionType
OP = mybir.AluOpType


def _build_program():
    nc = bacc_mod.Bacc("TRN2", target_bir_lowering=False)

    vT = nc.declare_dram_parameter("vT", [E, B], BF16, isOutput=False)
    nT = nc.declare_dram_parameter("nT", [E, B], BF16, isOutput=False)
    wat1 = nc.declare_dram_parameter("wat1", [E, E], BF16, isOutput=False)  # (wo@wv).T
    wat2 = nc.declare_dram_parameter("wat2", [E, E], BF16, isOutput=False)
    bat1 = nc.declare_dram_parameter("bat1", [E], F32, isOutput=False)
    bat2 = nc.declare_dram_parameter("bat2", [E], F32, isOutput=False)
    lnvg = nc.declare_dram_parameter("lnvg", [E], F32, isOutput=False)
    lnvb = nc.declare_dram_parameter("lnvb", [E], F32, isOutput=False)
    lnng = nc.declare_dram_parameter("lnng", [E], F32, isOutput=False)
    lnnb = nc.declare_dram_parameter("lnnb", [E], F32, isOutput=False)
    w1v = nc.declare_dram_parameter("w1v", [E, H2], BF16, isOutput=False)   # fv_w1.T
    b1v = nc.declare_dram_parameter("b1v", [H2], F32, isOutput=False)
    w2v = nc.declare_dram_parameter("w2v", [H2, E], BF16, isOutput=False)   # fv_w2.T
    b2v = nc.declare_dram_parameter("b2v", [E], F32, isOutput=False)
    w1n = nc.declare_dram_parameter("w1n", [E, H2], BF16, isOutput=False)
    b1n = nc.declare_dram_parameter("b1n", [H2], F32, isOutput=False)
    w2n = nc.declare_dram_parameter("w2n", [H2, E], BF16, isOutput=False)
    b2n = nc.declare_dram_parameter("b2n", [E], F32, isOutput=False)
    ones_f = nc.declare_dram_parameter("ones_f", [P, 1], F32, isOutput=False)
    ones_fr_d = nc.declare_dram_parameter("ones_fr_d", [1, P], F32, isOutput=False)
    ones_b_d = nc.declare_dram_parameter("ones_b_d", [P, 1], BF16, isOutput=False)
    verb_out = nc.declare_dram_parameter("verb_out", [E, B], F32, isOutput=True)
    noun_out = nc.declare_dram_parameter("noun_out", [E, B], F32, isOutput=True)

    with tile.TileContext(nc) as tc, ExitStack() as ctx:
        const = ctx.enter_context(tc.tile_pool(name="const", bufs=1))
        kxp = ctx.enter_context(tc.tile_pool(name="kxp", bufs=2))
        rp = ctx.enter_context(tc.tile_pool(name="rp", bufs=2))
        v1p = ctx.enter_context(tc.tile_pool(name="v1p", bufs=1))
        v2p = ctx.enter_context(tc.tile_pool(name="v2p", bufs=1))
        hp = ctx.enter_context(tc.tile_pool(name="hp", bufs=2))
        wap = ctx.enter_context(tc.tile_pool(name="wap", bufs=1))
        w1p = ctx.enter_context(tc.tile_pool(name="w1p", bufs=1))
        w2sp = ctx.enter_context(tc.tile_pool(name="w2sp", bufs=3))
        stp = ctx.enter_context(tc.tile_pool(name="stp", bufs=4))
        sqp = ctx.enter_context(tc.tile_pool(name="sqp", bufs=1))
        smp = ctx.enter_context(tc.tile_pool(name="smp", bufs=1))
        psp = ctx.enter_context(tc.tile_pool(name="psp", bufs=1, space="PSUM"))

        def psum(i):
            return psp.tile([P, CHUNK], F32, tag=f"b{i}", name=f"b{i}")

        def psum_stat(i):
            return psp.tile([1, CHUNK], F32, tag=f"b{i}", name=f"b{i}")

        ones_bf = const.tile([P, 1], BF16, tag="ones_bf", name="ones_bf")
        nc.sync.dma_start(out=ones_bf[:], in_=ones_b_d[:, :])
        ones_fr = const.tile([P, 1], F32R, tag="ones_fr", name="ones_fr")
        nc.sync.dma_start(out=ones_fr[:], in_=ones_f[:, :].bitcast(F32R))
        onesrow_fr = const.tile([1, P], F32R, tag="onesrow", name="onesrow")
        nc.sync.dma_start(out=onesrow_fr[:], in_=ones_fr_d[:, :].bitcast(F32R))
        eps_t = const.tile([1, 1], F32, tag="eps", name="eps")
        nc.vector.memset(eps_t[:], EPS)

        def load_pvec(dram_ap, ntiles, tag):
            t = const.tile([P, ntiles], F32, tag=tag, name=tag)
            nc.sync.dma_start(out=t[:], in_=dram_ap.rearrange("(t p) -> p t", p=P))
            return t

        bat1_pb = load_pvec(bat1[:], MT, "bat1")
        bat2_pb = load_pvec(bat2[:], MT, "bat2")
        lnvg_pb = load_pvec(lnvg[:], MT, "lnvg")
        lnvb_pb = load_pvec(lnvb[:], MT, "lnvb")
        lnng_pb = load_pvec(lnng[:], MT, "lnng")
        lnnb_pb = load_pvec(lnnb[:], MT, "lnnb")
        b1v_pb = load_pvec(b1v[:], HT, "b1v")
        b2v_pb = load_pvec(b2v[:], MT, "b2v")
        b1n_pb = load_pvec(b1n[:], HT, "b1n")
        b2n_pb = load_pvec(b2n[:], MT, "b2n")

        def load_attn_w(w_dram):
            tiles = []
            for k in range(KT):
                t = wap.tile([P, E], BF16, tag=f"aw{k}", name=f"aw{k}")
                nc.sync.dma_start(out=t[:], in_=w_dram[k * P:(k + 1) * P, :])
                tiles.append(t)
            return tiles

        def load_w1(w_dram):
            tiles = []
            for k in range(KT):
                t = w1p.tile([P, H2], BF16, tag=f"w1_{k}", name=f"w1_{k}")
                nc.sync.dma_start(out=t[:], in_=w_dram[k * P:(k + 1) * P, :])
                tiles.append(t)
            return tiles

        def attn_ln(wt, kx_dram, kx_tiles, res_dram, bias_pb, g_pb, b_pb,
                    out_tiles):
            """out[m][:,cs] = LN(res + W@kx + bias) for each 512-col chunk.

            Software-pipelined: chunk c's LN scale/shift is emitted after
            chunk c+1's matmuls so the DVE queue never head-of-line blocks
            the PSUM drains the PE is waiting on.
            """
            def bcast_apply(c, nm, rs):
                """Broadcast -mean/1/std across partitions, scale+shift."""
                cs = slice(c * CHUNK, (c + 1) * CHUNK)
                nmB = psum(6)
                nc.tensor.matmul(nmB[:], lhsT=onesrow_fr[:], rhs=nm[:],
                                 start=True, stop=True)
                rsB = psum(7)
                nc.tensor.matmul(rsB[:], lhsT=onesrow_fr[:], rhs=rs[:],
                                 start=True, stop=True)
                for m in range(MT):
                    xt = out_tiles[m][:, cs]
                    nc.vector.tensor_add(xt, xt, nmB[:])
                    nc.vector.tensor_mul(xt, xt, rsB[:])
                    nc.vector.tensor_scalar(
                        xt, xt, g_pb[:, m:m + 1], b_pb[:, m:m + 1],
                        OP.mult, OP.add)

            pending = None
            for c in range(NCH):
                cs = slice(c * CHUNK, (c + 1) * CHUNK)
                if kx_tiles is None:
                    kx = []
                    for k in range(KT):
                        t = kxp.tile([P, CHUNK], BF16, tag=f"kx{k}", name=f"kx{k}")
                        nc.sync.dma_start(out=t[:], in_=kx_dram[k * P:(k + 1) * P, cs])
                        kx.append(t[:])
                else:
                    kx = [kx_tiles[k][:, cs] for k in range(KT)]
                rt = []
                for m in range(MT):
                    t = rp.tile([P, CHUNK], BF16, tag=f"r{m}", name=f"r{m}")
                    nc.sync.dma_start(out=t[:], in_=res_dram[m * P:(m + 1) * P, cs])
                    rt.append(t)
                # stats matmuls run one m-iteration behind the mains so the
                # PE never waits on the per-m PSUM drain chain mid-stream
                stats_x = psum_stat(4)
                stats_q = psum_stat(5)
                sqs = []

                def stats(m):
                    nc.tensor.matmul(stats_x[:], lhsT=ones_bf[:],
                                     rhs=out_tiles[m][:, cs],
                                     start=(m == 0), stop=(m == MT - 1))
                    nc.tensor.matmul(stats_q[:], lhsT=ones_fr[:],
                                     rhs=sqs[m][:],
                                     start=(m == 0), stop=(m == MT - 1))

                for m in range(MT):
                    ps = psum(m % 4)
                    for k in range(KT):
                        nc.tensor.matmul(
                            ps[:], lhsT=wt[k][:, m * P:(m + 1) * P],
                            rhs=kx[k],
                            start=(k == 0), stop=(k == KT - 1))
                    xt = out_tiles[m][:, cs]
                    nc.vector.tensor_scalar(
                        xt, ps[:], bias_pb[:, m:m + 1], None, OP.add)
                    nc.vector.tensor_add(xt, xt, rt[m][:])
                    sqm = sqp.tile([P, CHUNK], F32R, tag=f"sq{m % 2}",
                                   name=f"sq{m % 2}")
                    nc.scalar.activation(sqm[:], xt, AF.Square)
                    sqs.append(sqm)
                    if m > 0:
                        stats(m - 1)
                stats(MT - 1)
                # column stats -> -mean, 1/std  ([1, CHUNK])
                nm = smp.tile([1, CHUNK], F32R, tag=f"nm{c % 2}", name=f"nm{c % 2}")
                nc.scalar.activation(nm[:], stats_x[:], AF.Copy, scale=-1.0 / E)
                t1 = smp.tile([1, CHUNK], F32, tag="t1", name="t1")
                nc.scalar.activation(t1[:], stats_q[:], AF.Copy, scale=1.0 / E)
                m2 = smp.tile([1, CHUNK], F32, tag="m2", name="m2")
                nc.vector.tensor_mul(m2[:], nm[:], nm[:])
                nc.vector.tensor_sub(t1[:], t1[:], m2[:])           # var
                nc.scalar.activation(t1[:], t1[:], AF.Sqrt, bias=eps_t[:])
                rs = smp.tile([1, CHUNK], F32R, tag=f"rs{c % 2}", name=f"rs{c % 2}")
                with nc.allow_low_precision(reason="f32r is bit-identical to f32"):
                    nc.vector.reciprocal(rs[:], t1[:])
                if pending is not None:
                    bcast_apply(*pending)
                pending = (c, nm, rs)
            bcast_apply(*pending)

        def ffn(in_tiles, w1t, b1_pb, w2_dram, b2_pb, out_dram, out_bf):
            """out = in + W2.T@gelu(W1.T@in + b1) + b2, per 512-col chunk."""
            for c in range(NCH):
                cs = slice(c * CHUNK, (c + 1) * CHUNK)
                hts = []
                for hm in range(HT):
                    ps = psum(hm % 4)
                    for k in range(KT):
                        nc.tensor.matmul(
                            ps[:], lhsT=w1t[k][:, hm * P:(hm + 1) * P],
                            rhs=in_tiles[k][:, cs],
                            start=(k == 0), stop=(k == KT - 1))
                    ht = hp.tile([P, CHUNK], BF16, tag=f"h{hm}", name=f"h{hm}")
                    nc.scalar.activation(ht[:], ps[:], AF.Gelu,
                                         bias=b1_pb[:, hm:hm + 1])
                    hts.append(ht)
                for blk in range(2):
                    ms = range(blk * 4, blk * 4 + 4)
                    pss = [psum(blk * 4 + mi) for mi in range(4)]
                    for k in range(HT):
                        w2t = w2sp.tile([P, CHUNK], BF16, tag="w2s", name="w2s")
                        nc.sync.dma_start(
                            out=w2t[:],
                            in_=w2_dram[k * P:(k + 1) * P,
                                        blk * CHUNK:(blk + 1) * CHUNK])
                        for mi, m in enumerate(ms):
                            nc.tensor.matmul(
                                pss[mi][:], lhsT=w2t[:, mi * P:(mi + 1) * P],
                                rhs=hts[k][:],
                                start=(k == 0), stop=(k == HT - 1))
                    for mi, m in enumerate(ms):
                        st = stp.tile([P, CHUNK], F32, tag="st", name="st")
                        nc.vector.tensor_add(st[:], pss[mi][:],
                                             in_tiles[m][:, cs])
                        nc.scalar.activation(st[:], st[:], AF.Identity,
                                             bias=b2_pb[:, m:m + 1])
                        nc.sync.dma_start(out=out_dram[m * P:(m + 1) * P, cs],
                                          in_=st[:])
                        if out_bf is not None:
                            nc.scalar.activation(out_bf[m][:, cs], st[:],
                                                 AF.Copy)

        _REP = int(os.environ.get("BENCH_REPEAT", "1"))
        for _rep in range(_REP):
            # A: verb attends to noun, LN -> verb1 (SBUF resident)
            wA = load_attn_w(wat1)
            v1 = [v1p.tile([P, B], BF16, tag=f"v1_{m}", name=f"v1_{m}")
                  for m in range(MT)]
            attn_ln(wA, nT, None, vT, bat1_pb, lnvg_pb, lnvb_pb, v1)
            w1tv = load_w1(w1v)
            # B: verb FFN -> verb_out (DRAM, f32) + verb2 (SBUF bf16)
            v2 = [v2p.tile([P, B], BF16, tag=f"v2_{m}", name=f"v2_{m}")
                  for m in range(MT)]
            ffn(v1, w1tv, b1v_pb, w2v, b2v_pb, verb_out, v2)
            # C: noun attends to verb2 (SBUF), LN -> noun1 (reuses v1 slots)
            wC = load_attn_w(wat2)
            w1tn = load_w1(w1n)
            n1 = [v1p.tile([P, B], BF16, tag=f"v1_{m}", name=f"v1_{m}")
                  for m in range(MT)]
            attn_ln(wC, None, v2, nT, bat2_pb, lnng_pb, lnnb_pb, n1)
            # D: noun FFN -> noun_out
            ffn(n1, w1tn, b1n_pb, w2n, b2n_pb, noun_out, None)

    nc.finalize()
    return nc


_prog_cache = {}


def _get_program():
    if "nc" not in _prog_cache:
        _prog_cache["nc"] = _build_program()
    return _prog_cache["nc"]


def _prepare_maps(inputs):
    f32 = np.float32
    bf16 = ml_dtypes.bfloat16
    g = {k: np.asarray(v, f32) for k, v in inputs.items()}

    def fold(p):
        w = g[f"{p}_wo"] @ g[f"{p}_wv"]
        b = g[f"{p}_wo"] @ g[f"{p}_bv"] + g[f"{p}_bo"]
        return np.ascontiguousarray(w.T).astype(bf16), np.ascontiguousarray(b)

    wat1, bat1 = fold("v2n")
    wat2, bat2 = fold("n2v")
    common = {
        "wat1": wat1, "bat1": bat1, "wat2": wat2, "bat2": bat2,
        "lnvg": g["ln_v_g"], "lnvb": g["ln_v_b"],
        "lnng": g["ln_n_g"], "lnnb": g["ln_n_b"],
        "w1v": np.ascontiguousarray(g["fv_w1"].T).astype(bf16), "b1v": g["fv_b1"],
        "w2v": np.ascontiguousarray(g["fv_w2"].T).astype(bf16), "b2v": g["fv_b2"],
        "w1n": np.ascontiguousarray(g["fn_w1"].T).astype(bf16), "b1n": g["fn_b1"],
        "w2n": np.ascontiguousarray(g["fn_w2"].T).astype(bf16), "b2n": g["fn_b2"],
        "ones_f": np.ones((P, 1), f32),
        "ones_fr_d": np.ones((1, P), f32),
        "ones_b_d": np.ones((P, 1), bf16),
    }
    vT = np.ascontiguousarray(g["verb_features"].T).astype(bf16)  # [E, 16384]
    nT = np.ascontiguousarray(g["noun_features"].T).astype(bf16)
    in_maps = []
    for i in range(NCORES):
        cs = slice(i * B, (i + 1) * B)
        m = dict(common)
        m["vT"] = np.ascontiguousarray(vT[:, cs])
        m["nT"] = np.ascontiguousarray(nT[:, cs])
        in_maps.append(m)
    return in_maps


def kernel(**inputs):
    nc = _get_program()
    in_maps = _prepare_maps(inputs)
    res = run_bass_kernel_spmd(nc, in_maps, list(range(NCORES))).results
    verb = np.concatenate([res[i]["verb_out"] for i in range(NCORES)], axis=1)
    noun = np.concatenate([res[i]["noun_out"] for i in range(NCORES)], axis=1)
    return np.ascontiguousarray(verb.T), np.ascontiguousarray(noun.T)
